# revision 12
# baseline (speedup 1.0000x reference)
# kernel.py — 3x3 avg-pool (stride 1, VALID) over NCHW f32 via Bass/Tile on 8 TRN2 cores.
#
# Layout: one image per SBUF partition, [64, 64] in the free dim.  Both pooling
# directions are then free-dim shifted adds (separable 3-tap x 3-tap):
#   H-pass: mid[r, w] = x[r,w] + x[r,w+1] + x[r,w+2]       (2 adds)
#   V-pass: sum[r, w] = mid[r,w] + mid[r+1,w] + mid[r+2,w] (2 adds)
#   scale:  out = sum * (1/9)                              (ScalarE, in-place)
# fp32 tensor_tensor runs at 1x on DVE, so rows are split across engines.
# A hardware race-oracle probe (probe_timing.py) measured GPSIMD tensor_add at
# ~1.75x DVE's per-element time, putting the balanced split at DVE 39 rows /
# GPSIMD 23 rows.  GPSIMD takes the TOP rows: they arrive in the first chunk
# of the (split) tile-0 load, so the slower engine starts ~4 us earlier.
# fp32 TT never enters a 2-port DVE perf mode, so DVE and GPSIMD don't contend
# for the shared SBUF port pair; the 1/9 runs on ScalarE (own ports, measured
# 2x mode for fp32 copies).  The PE is useless here: fp32 matmul is 4 cyc/row
# and HW-inexact (probed, abs err ~6.5 on an identity matmul); fp32r ~12-bit.
#
# DMA: flat layout gives perfectly contiguous per-partition descriptors.
# Pools use bufs=NT (no slot reuse) so DMAs need at most one sync-wait;
# remaining multi-wait instructions (Tile's kernel-tail drain) are legalized
# by _split_multiwait, since this walrus allows one embedded sync-wait per
# instruction.  Tile 0's load is split three ways (GPSIMD rows first, then
# DVE rows in two chunks feeding a chunked H-pass) to cut pipeline lead-in;
# tile 3's store is split across the two HWDGE rings (SP + ACT) for the tail.
#
# Full input (16, 256, 64, 64) is sharded 4096 images -> 8 cores x 512 images
# (contiguous N*C ranges), no cross-core communication.

import numpy as np

N_CORES = 8
N, C = 16, 256
H = W = 64
OH = OW = 62
P = 128                        # SBUF partitions = images per mega-tile
IMGS_PER_CORE = (N * C) // N_CORES    # 512
NT = IMGS_PER_CORE // P        # 4 mega-tiles per core

VG = 23                        # output rows 0..VG-1 on GPSIMD (top)
                               # output rows VG..61 on DVE (bottom)

_nc_cache = {}


def _split_multiwait(nc, max_waits=1):
    """Walrus's codegen allows only one embedded sync-wait per instruction
    (HW-decode struct limit); Tile's kernel-tail drain carries the whole
    global clock.  Move excess waits onto single-wait EventSemaphore
    instructions inserted immediately before the offending instruction on
    the same engine."""
    import concourse.mybir as mb

    for f in nc.m.functions:
        for b in f.blocks:
            new_list = []
            for inst in b.instructions:
                si = getattr(inst, "sync_info", None)
                if si is not None and len(si.on_wait) > max_waits:
                    waits = list(si.on_wait)
                    extra, keep = waits[:-max_waits], waits[-max_waits:]
                    for k, w in enumerate(extra):
                        es = mb.InstEventSemaphore(
                            name=f"{inst.name}-esw{k}", ins=[], outs=[],
                            engine=inst.engine)
                        es.sync_info = mb.SyncInfo(on_wait=[w], on_update=[])
                        nc.register_instruction(es)
                        new_list.append(es)
                    inst.sync_info = mb.SyncInfo(
                        on_wait=keep, on_update=list(si.on_update))
                new_list.append(inst)
            b.instructions[:] = new_list


def _build_nc(vg=VG):
    import concourse.bass as bass
    import concourse.mybir as mybir
    from concourse.tile import TileContext

    f32 = mybir.dt.float32

    nc = bass.Bass()
    x = nc.declare_dram_parameter("x", [IMGS_PER_CORE, H, W], f32, isOutput=False)
    o = nc.declare_dram_parameter("o", [IMGS_PER_CORE, OH, OW], f32, isOutput=True)

    vd = OH - vg       # DVE output rows vg..61
    hg = vg + 2        # GPSIMD mid rows 0..hg-1   (needs x rows 0..hg-1)
    hd = H - vg        # DVE mid rows vg..63       (needs x rows vg..63)
    mb0 = 45           # tile-0 DVE H chunk boundary (x-row index)

    with TileContext(nc) as tc:
        with (
            tc.tile_pool(name="xp", bufs=NT) as xp,
            tc.tile_pool(name="md", bufs=NT) as md,
            tc.tile_pool(name="mg", bufs=NT) as mg,
            tc.tile_pool(name="op", bufs=NT) as op,
        ):
            for t in range(NT):
                xt = xp.tile([P, H, W], f32)
                if t == 0:
                    # GPSIMD's rows first, then DVE's rows in two chunks:
                    # both engines start as soon as their data lands.
                    nc.sync.dma_start(out=xt[:, 0:hg, :], in_=x[0:P, 0:hg])
                    nc.sync.dma_start(out=xt[:, hg:mb0, :], in_=x[0:P, hg:mb0])
                    nc.sync.dma_start(out=xt[:, mb0:H, :], in_=x[0:P, mb0:H])
                else:
                    nc.sync.dma_start(out=xt[:], in_=x[t * P:(t + 1) * P])

                # ---- GPSIMD chain: out rows 0..vg-1 (top) ----
                mid_g = mg.tile([P, hg, OW], f32)
                nc.gpsimd.tensor_add(
                    out=mid_g[:], in0=xt[:, 0:hg, 0:62], in1=xt[:, 0:hg, 1:63])
                nc.gpsimd.tensor_add(
                    out=mid_g[:], in0=mid_g[:], in1=xt[:, 0:hg, 2:64])

                # ---- DVE chain: out rows vg..61 (bottom) ----
                mid_d = md.tile([P, hd, OW], f32)
                if t == 0:
                    ca = mb0 - vg     # H rows in chunk A (x rows vg..mb0-1)
                    nc.vector.tensor_add(
                        out=mid_d[:, 0:ca, :],
                        in0=xt[:, vg:mb0, 0:62], in1=xt[:, vg:mb0, 1:63])
                    nc.vector.tensor_add(
                        out=mid_d[:, 0:ca, :],
                        in0=mid_d[:, 0:ca, :], in1=xt[:, vg:mb0, 2:64])
                    nc.vector.tensor_add(
                        out=mid_d[:, ca:hd, :],
                        in0=xt[:, mb0:H, 0:62], in1=xt[:, mb0:H, 1:63])
                    nc.vector.tensor_add(
                        out=mid_d[:, ca:hd, :],
                        in0=mid_d[:, ca:hd, :], in1=xt[:, mb0:H, 2:64])
                else:
                    nc.vector.tensor_add(
                        out=mid_d[:], in0=xt[:, vg:H, 0:62], in1=xt[:, vg:H, 1:63])
                    nc.vector.tensor_add(
                        out=mid_d[:], in0=mid_d[:], in1=xt[:, vg:H, 2:64])

                ot = op.tile([P, OH, OW], f32)
                nc.gpsimd.tensor_add(
                    out=ot[:, 0:vg, :], in0=mid_g[:, 0:vg, :], in1=mid_g[:, 1:vg + 1, :])
                nc.gpsimd.tensor_add(
                    out=ot[:, 0:vg, :], in0=ot[:, 0:vg, :], in1=mid_g[:, 2:vg + 2, :])
                nc.vector.tensor_add(
                    out=ot[:, vg:OH, :], in0=mid_d[:, 0:vd, :], in1=mid_d[:, 1:vd + 1, :])
                nc.vector.tensor_add(
                    out=ot[:, vg:OH, :], in0=ot[:, vg:OH, :], in1=mid_d[:, 2:vd + 2, :])

                # ---- 1/9 on ScalarE (own SBUF ports; never contends) ----
                nc.scalar.mul(out=ot[:, 0:vg, :], in_=ot[:, 0:vg, :], mul=1.0 / 9.0)
                nc.scalar.mul(out=ot[:, vg:OH, :], in_=ot[:, vg:OH, :], mul=1.0 / 9.0)

                if t == NT - 1:
                    # split last store across both HWDGE rings: shorter tail
                    nc.scalar.dma_start(
                        out=o[t * P:(t + 1) * P, 0:vg, :], in_=ot[:, 0:vg, :])
                    nc.sync.dma_start(
                        out=o[t * P:(t + 1) * P, vg:OH, :], in_=ot[:, vg:OH, :])
                else:
                    nc.sync.dma_start(out=o[t * P:(t + 1) * P], in_=ot[:])

    _split_multiwait(nc)
    nc.finalize()
    return nc


def _get_nc(vg=VG):
    if vg not in _nc_cache:
        _nc_cache[vg] = _build_nc(vg)
    return _nc_cache[vg]


def run(x, trace=False, vg=VG, **spmd_kwargs):
    """Run the pool kernel on 8 cores. x: (16,256,64,64) f32. Returns
    (output (16,256,62,62) f32, BassKernelResults)."""
    from concourse.bass_utils import run_bass_kernel_spmd

    x = np.ascontiguousarray(np.asarray(x, dtype=np.float32))
    assert x.shape == (N, C, H, W), x.shape
    shards = x.reshape(N_CORES, IMGS_PER_CORE, H, W)
    in_maps = [{"x": shards[c]} for c in range(N_CORES)]
    nc = _get_nc(vg)
    res = run_bass_kernel_spmd(
        nc, in_maps, list(range(N_CORES)), trace=trace, **spmd_kwargs
    )
    out = np.stack([res.results[c]["o"] for c in range(N_CORES)], axis=0)
    return out.reshape(N, C, OH, OW), res


def kernel(x):
    out, _ = run(x, trace=False)
    return out
